# revision 49
# baseline (speedup 1.0000x reference)
"""AttentionPooling kernel for TRN2 (8 NeuronCores, data-parallel over batch).

Reference computation (per batch b, per span n):
  x = token_reps + sinusoidal_pe                     (S, H)
  window = [start_n, end_n)  (width <= 32, all indices in-range)
  q shared across spans; per-head scores over the window -> softmax -> pool V
  attn_out = ctx @ Wo^T + out_b; y1 = LN(attn_out + dq)
  y = LN(relu(y1@W1^T+b1)@W2^T+b2 + y1); zero masked spans

Reformulation (no gather): per-token scores ts[s,h] = x[s].(scale*Wk_h^T q_h)
are shared across spans; softmax over a span's window + pooling of
v_tok = x @ Wv^T becomes masked matmuls with the 0/1 window matrix M[s,n]
(built on HOST and DMAed):
  ctxT[:,n] = (ev^T @ M)[:,n] / (e^T @ M)[head,n],  ev[s,:] = e[s,head]*v_tok[s]
The swapped operand order (lhsT=ev, rhs=M) yields ctxT (h on partitions)
directly -- no PE transpose of ctx needed.  The per-(head,n) reciprocal
denominator r[4,N] is row-broadcast to [128,N] via a tiny constant sel-matmul
and fused into the PSUM evacuation (tensor_tensor multiply).

x is transposed on the HOST, so the x-transpose stage disappears too.
attn bias (out_b + Wo@bv + dq) is folded in via a K=1 ones-row matmul.

LayerNorm scale-invariance: y1 is kept as 256*LN(ao) in fp16.  FFN1 output is
evacuated as 256*relu(y1@W1+b1) (scale folded into the relu), FFN2 output is
then 256*ff2 and the final LN sees 256*(ff2+y1) == same output.  This makes
the y1 transposes fp16 and lets y1T/W1 quantize to fp8e4 for a DoubleRow
matmul (2x PE rate) with all scales absorbed for free.

Pacing: every PE idle gap triggers a multi-us HAM half-clock window, so the
kernel keeps the PE queue dense: warm-up transposes run on an uninitialized
tile (no input deps), PSUM pools are triple-buffered where LN/evac chains
lag, LN applies run on the vector engine as a single tensor_scalar, and the
y1 transposes are interleaved between the stage-F matmul groups.
"""

import numpy as np
import ml_dtypes

import concourse.bass as bass
import concourse.bacc as bacc
import concourse.mybir as mybir
import concourse.tile as tile
from concourse.bass_utils import run_bass_kernel_spmd
from concourse.masks import make_identity

B, S, N, H = 8, 512, 512, 768
NH, HD = 4, 192
F = 3072
NT = S // 128   # 4  partition tiles over s or n
KH = H // 128   # 6  partition tiles over hidden dim
KF = F // 128   # 24 partition tiles over ffn dim
f32 = mybir.dt.float32
bf16 = mybir.dt.float16  # fp16: same PE rate as bf16, 8x the mantissa
fp8 = mybir.dt.float8e4
AF = mybir.ActivationFunctionType
OP = mybir.AluOpType
PM = mybir.MatmulPerfMode
BF = np.float16
F8 = ml_dtypes.float8_e4m3fn

Y1S = 256.0        # y1 carried as Y1S*LN(ao); LN2 is scale-invariant
W1SC = 64.0        # fp8 W1 pre-scale (values ~0.02 -> ~1.3, away from subnorms)
Y1Q = 16.0         # fp8 y1 pre-scale (|16*y1| < ~80 << 240 TRN e4m3 max)


def _mm(nc, out, lhsT, rhs, start, stop, perf_mode=None):
    nc.tensor.matmul(out, lhsT, rhs, start=start, stop=stop, perf_mode=perf_mode)


def _bcast_row(nc, dst, handle, n):
    # DMA-broadcast a length-n DRAM vector across 128 partitions.
    nc.sync.dma_start(out=dst, in_=bass.AP(handle, 0, [[0, 128], [1, n]]))


def build_bass(apply_gb=False, apply_b2=False, apply_b1=False, apply_mask=False,
               use_fp8=True):
    from contextlib import ExitStack

    nc = bacc.Bacc("TRN2", target_bir_lowering=False, debug=False)

    xt_d = nc.dram_tensor("xt", [H, S], bf16, kind="ExternalInput")
    m_d = nc.dram_tensor("m", [S, N], bf16, kind="ExternalInput")
    wvus_d = nc.dram_tensor("wvus", [H, H + 4], bf16, kind="ExternalInput")
    wot_d = nc.dram_tensor("wot", [H, H], bf16, kind="ExternalInput")
    addvr_d = nc.dram_tensor("addvr", [1, H], bf16, kind="ExternalInput")
    sel_d = nc.dram_tensor("sel", [NH, H], bf16, kind="ExternalInput")
    if use_fp8:
        w1t_d = nc.dram_tensor("w1t8", [H, F], fp8, kind="ExternalInput")
    else:
        w1t_d = nc.dram_tensor("w1t", [H, F], bf16, kind="ExternalInput")
    w2t_d = nc.dram_tensor("w2t", [F, H], bf16, kind="ExternalInput")
    out_d = nc.dram_tensor("out", [N, H], f32, kind="ExternalOutput")
    if apply_mask:
        maskc_d = nc.dram_tensor("maskc", [128, NT], f32, kind="ExternalInput")
    if apply_b1:
        b1c_d = nc.dram_tensor("b1c", [128, KF], f32, kind="ExternalInput")
    if apply_b2:
        b2_d = nc.dram_tensor("b2", [H], f32, kind="ExternalInput")
    if apply_gb:
        lng_d = nc.dram_tensor("lng", [H], f32, kind="ExternalInput")
        lnb_d = nc.dram_tensor("lnb", [H], f32, kind="ExternalInput")
        lnb256_d = nc.dram_tensor("lnb256", [H], f32, kind="ExternalInput")

    out_ap = out_d.ap()

    with tile.TileContext(nc) as tc:
        with (
            tc.tile_pool(name="singles", bufs=1) as singles,
            tc.tile_pool(name="y1p", bufs=1) as y1p,
            tc.tile_pool(name="w1p", bufs=1) as w1p,
            tc.tile_pool(name="w2p", bufs=1) as w2p,
            tc.tile_pool(name="ffp", bufs=1) as ffp,
            tc.tile_pool(name="outp", bufs=2) as outp,
            tc.tile_pool(name="lnp", bufs=2) as lnp,
        ):
            # HAM warm-up on an UNINITIALIZED tile: no input deps, so the PE
            # starts within ~0.5us and stays hot until the input DMAs land.
            junk = singles.tile([128, 128], bf16)
            nc.vector.memset(junk, 0.0)
            with tc.tile_pool(name="psW", bufs=1, space="PSUM") as psW:
                wps = psW.tile([128, S], f32, tag="wps")
                for r in range(8):
                    for c in range(NT):
                        nc.tensor.matmul(
                            wps[:, c * 128 : (c + 1) * 128], junk, junk,
                            start=True, stop=True,
                        )

            ident_bf = singles.tile([128, 128], bf16)
            make_identity(nc, ident_bf)
            ones1 = singles.tile([1, 128], bf16)
            nc.vector.memset(ones1, 1.0)
            eps_t = singles.tile([128, 1], f32)
            nc.vector.memset(eps_t, 1e-5)
            # eps/Y1S^2: folds the Y1S factor into sqrt's pre-scale, so
            # reciprocal directly yields Y1S*rstd (one fewer chain hop)
            eps2_t = singles.tile([128, 1], f32)
            nc.vector.memset(eps2_t, 1e-5 / (Y1S * Y1S))

            # --- input DMAs (sync queue) in consumption order ---
            xt = singles.tile([128, KH, S], bf16)
            xt_r = xt_d.ap().rearrange("(t p) s -> p t s", p=128)
            wv = singles.tile([128, KH, H + 4], bf16)
            wv_r = wvus_d.ap().rearrange("(t p) h -> p t h", p=128)
            mt_t = singles.tile([128, NT, N], bf16)
            m_r = m_d.ap().rearrange("(t p) n -> p t n", p=128)
            for kt in range(KH):
                nc.sync.dma_start(out=xt[:, kt, :], in_=xt_r[:, kt, :])
                nc.sync.dma_start(out=wv[:, kt, :], in_=wv_r[:, kt, :])
                if kt == 2:
                    nc.sync.dma_start(out=mt_t, in_=m_r)
            wotr = singles.tile([128, KH, H], bf16)
            nc.sync.dma_start(
                out=wotr, in_=wot_d.ap().rearrange("(t p) h -> p t h", p=128)
            )
            addvr = singles.tile([1, H], bf16)
            nc.sync.dma_start(out=addvr, in_=addvr_d.ap())
            sel_t = singles.tile([NH, H], bf16)
            nc.sync.dma_start(out=sel_t, in_=sel_d.ap())

            maskc_t = b1c_t = b2_b = g_b = b_b = b256_b = None
            if apply_mask:
                maskc_t = singles.tile([128, NT], f32)
                nc.sync.dma_start(out=maskc_t, in_=maskc_d.ap())
            if apply_b1:
                b1c_t = singles.tile([128, KF], f32)
                nc.sync.dma_start(out=b1c_t, in_=b1c_d.ap())
            if apply_b2:
                b2_b = singles.tile([128, H], f32)
                _bcast_row(nc, b2_b, b2_d, H)
            if apply_gb:
                g_b = singles.tile([128, H], f32)
                _bcast_row(nc, g_b, lng_d, H)
                b_b = singles.tile([128, H], f32)
                _bcast_row(nc, b_b, lnb_d, H)
                b256_b = singles.tile([128, H], f32)
                _bcast_row(nc, b256_b, lnb256_d, H)

            # big FFN weights stream behind the attention inputs
            w1r = w1p.tile([128, KH, F], fp8 if use_fp8 else bf16)
            nc.sync.dma_start(
                out=w1r, in_=w1t_d.ap().rearrange("(t p) h -> p t h", p=128)
            )
            w2r = w2p.tile([128, KF, H], bf16)
            nc.sync.dma_start(
                out=w2r, in_=w2t_d.ap().rearrange("(t p) h -> p t h", p=128)
            )

            # dep-free PE filler: keeps the clock governor at full rate
            # through structural waits (each PE idle gap otherwise triggers
            # a multi-us half-clock window)
            es_psJ = ExitStack()
            psJ = es_psJ.enter_context(tc.tile_pool(name="psJ", bufs=1, space="PSUM"))
            jps = psJ.tile([128, 128], f32, tag="jps")

            def filler(n):
                for _ in range(n):
                    nc.tensor.matmul(jps, junk, junk, start=True, stop=True)

            es_ctx = ExitStack()
            ctxTp = es_ctx.enter_context(tc.tile_pool(name="ctxTp", bufs=1))
            es1 = ExitStack()
            evp = es1.enter_context(tc.tile_pool(name="evp", bufs=1))

            # --- stage B: v_tok | ts = xT.T @ [WvT | Us]; ev = e * v ---
            # ev evac split across scalar (exp, e-copy, heads 0-1) and vector
            # (heads 2-3) so ev[st] is ready ~1us after psv[st] stops.
            # e-cols live in their own small tiles so denT (which needs only
            # e) never waits on the wide v-scaling muls.
            es_psDen = ExitStack()
            psDen = es_psDen.enter_context(
                tc.tile_pool(name="psDen", bufs=1, space="PSUM"))
            denps = psDen.tile([NH, N], f32, tag="den")

            ev = []
            e4 = []
            es_psB = ExitStack()
            psB = es_psB.enter_context(tc.tile_pool(name="psB", bufs=2, space="PSUM"))
            for st in range(NT):
                psv = psB.tile([128, H + 4], f32, tag="psv", name=f"psB{st}")
                for kt in range(KH):
                    lhsT = xt[:, kt, st * 128 : (st + 1) * 128]
                    _mm(nc, psv[:, 0:512], lhsT, wv[:, kt, 0:512],
                        kt == 0, kt == KH - 1)
                    _mm(nc, psv[:, 512:772], lhsT, wv[:, kt, 512:772],
                        kt == 0, kt == KH - 1)
                if st >= 1:
                    # denT(st-1) interleaved into B: e4[st-1] is ready, so
                    # the den accumulation finishes (and the 3.3us vector
                    # reciprocal starts) right at B's end instead of after it
                    _mm(nc, denps, e4[st - 1], mt_t[:, st - 1, :],
                        st - 1 == 0, False)
                e_f = lnp.tile([128, 4], f32, tag="e_f", bufs=4)
                nc.scalar.activation(out=e_f, in_=psv[:, 768:772], func=AF.Exp)
                e4t = evp.tile([128, 4], bf16, tag=f"e4_{st}", name=f"e4_{st}")
                nc.scalar.copy(out=e4t, in_=e_f)
                e4.append(e4t)
                evt = evp.tile([128, H], bf16, tag=f"ev{st}", name=f"ev{st}")
                for h in range(NH):
                    # heads 0-1 on scalar, 2-3 on vector: halves each queue's
                    # chain so the recip (vector) starts sooner after B
                    if h < 2:
                        nc.scalar.activation(
                            out=evt[:, h * HD : (h + 1) * HD],
                            in_=psv[:, h * HD : (h + 1) * HD],
                            func=AF.Identity, scale=e_f[:, h : h + 1],
                        )
                    else:
                        nc.vector.tensor_scalar_mul(
                            out=evt[:, h * HD : (h + 1) * HD],
                            in0=psv[:, h * HD : (h + 1) * HD],
                            scalar1=e_f[:, h : h + 1],
                        )
                ev.append(evt)
            es_psB.close()

            # --- stage D': ctxT = ev^T @ M, normalized by r = 1/(e^T @ M) ---
            es_psD = ExitStack()
            psD = es_psD.enter_context(tc.tile_pool(name="psD", bufs=3, space="PSUM"))
            psR = es_psD.enter_context(tc.tile_pool(name="psR", bufs=2, space="PSUM"))
            psc = [psD.tile([128, N], f32, tag="psc", name=f"psD{j}")
                   for j in range(KH)]
            rbps = [psR.tile([128, N], f32, tag="rb", name=f"psR{j}")
                    for j in range(KH)]
            ctxT = [ctxTp.tile([128, N], bf16, tag=f"ctxT{j}", name=f"ctxT{j}")
                    for j in range(KH)]
            rb16 = [lnp.tile([128, N], bf16, tag=f"rb16_{j}", bufs=1,
                             name=f"rb16_{j}")
                    for j in range(KH)]

            def pscmm(j, sts):
                for st in sts:
                    _mm(nc, psc[j], ev[st][:, j * 128 : (j + 1) * 128],
                        mt_t[:, st, :], st == 0, st == NT - 1)

            def rbmm(j):
                _mm(nc, rbps[j], sel_t[:, j * 128 : (j + 1) * 128], r16,
                    True, True)
                if j % 2 == 0:
                    nc.scalar.copy(out=rb16[j], in_=rbps[j])
                else:
                    nc.vector.tensor_copy(out=rb16[j], in_=rbps[j])

            def ctevac(j):
                nc.vector.tensor_mul(out=ctxT[j], in0=psc[j], in1=rb16[j])

            # finish denT (st=3); psc j=0 matmuls cover the recip chain
            _mm(nc, denps, e4[NT - 1], mt_t[:, NT - 1, :], False, True)
            pscmm(0, range(3))
            if apply_mask:
                # masked spans have empty windows: clamp 0 -> tiny before 1/x
                nc.vector.tensor_scalar_max(out=denps, in0=denps, scalar1=1e-30)
            r16 = lnp.tile([NH, N], bf16, tag="r16")
            with nc.allow_low_precision(reason="1/den in fp16: 5e-4 rel, validated"):
                nc.vector.reciprocal(out=r16, in_=denps)
            pscmm(0, [3])
            pscmm(1, range(NT))
            rbmm(0)
            rbmm(1)
            ctevac(0)
            pscmm(2, range(NT))
            rbmm(2)
            rbmm(3)
            ctevac(1)
            pscmm(3, range(NT))
            rbmm(4)
            rbmm(5)
            ctevac(2)
            pscmm(4, range(NT))
            ctevac(3)
            pscmm(5, range(NT))
            ctevac(4)
            ctevac(5)
            filler(6)
            es_psD.close()
            es_psDen.close()
            es1.close()  # free ev

            # --- stage F: psa = ctxT.T @ WoT + 1^T.addv ; y1 = Y1S*LN(psa) ---
            # y1 transposes (stage G) are interleaved between F matmul groups
            # to cover the LN chain latency of the last row tiles.
            y1 = [y1p.tile([128, H], bf16, tag=f"y1_{i}", name=f"y1_{i}")
                  for i in range(NT)]
            es_psF = ExitStack()
            psF = es_psF.enter_context(tc.tile_pool(name="psF", bufs=2, space="PSUM"))
            es_psG = ExitStack()
            psG = es_psG.enter_context(tc.tile_pool(name="psG", bufs=1, space="PSUM"))
            # two jt-tiles packed per PSUM bank: [128, 2, 512] fp16 = 2KB
            psGt = [psG.tile([128, 2, S], bf16, tag=f"psG{p}", name=f"psG{p}")
                    for p in range(KH // 2)]

            def ln_stats(pool, in_ap, scaled=False):
                # mean/var over free dim (768); 2 bn_stats chunks of 384.
                # scaled=True: sqrt computes std/Y1S so recip gives Y1S*rstd.
                stats = pool.tile([128, 2, 6], f32, tag="ln_stats")
                for c in range(2):
                    nc.vector.bn_stats(out=stats[:, c, :],
                                       in_=in_ap[:, c * 384 : (c + 1) * 384])
                mv = pool.tile([128, 2], f32, tag="ln_mv")
                nc.vector.bn_aggr(out=mv, in_=stats)
                std = pool.tile([128, 1], f32, tag="ln_std")
                if scaled:
                    nc.scalar.activation(out=std, in_=mv[:, 1:2], func=AF.Sqrt,
                                         bias=eps2_t[:, 0:1],
                                         scale=1.0 / (Y1S * Y1S))
                else:
                    nc.scalar.activation(out=std, in_=mv[:, 1:2], func=AF.Sqrt,
                                         bias=eps_t[:, 0:1])
                rstd = pool.tile([128, 1], f32, tag="ln_rstd")
                nc.vector.reciprocal(out=rstd, in_=std)
                return mv, rstd

            def fstage(nt):
                psa = psF.tile([128, H], f32, tag="psa", name=f"psF{nt}")
                for kt in range(KH):
                    lhsT = ctxT[kt][:, nt * 128 : (nt + 1) * 128]
                    _mm(nc, psa[:, 0:512], lhsT, wotr[:, kt, 0:512],
                        kt == 0, False)
                    _mm(nc, psa[:, 512:768], lhsT, wotr[:, kt, 512:768],
                        kt == 0, False)
                _mm(nc, psa[:, 0:512], ones1, addvr[:, 0:512], False, True)
                _mm(nc, psa[:, 512:768], ones1, addvr[:, 512:768], False, True)
                mv, rs = ln_stats(lnp, psa, scaled=True)
                # y1 = (psa - mu) * (Y1S*rstd), single vector op, fp16 out
                nc.vector.tensor_scalar(out=y1[nt], in0=psa,
                                        scalar1=mv[:, 0:1], scalar2=rs[:, 0:1],
                                        op0=OP.subtract, op1=OP.mult)
                if apply_gb:
                    nc.vector.tensor_mul(out=y1[nt], in0=y1[nt], in1=g_b)
                    nc.vector.tensor_add(out=y1[nt], in0=y1[nt], in1=b256_b)

            def gpass(nts):
                for nt in nts:
                    for jt in range(KH):
                        nc.tensor.transpose(
                            psGt[jt // 2][:, jt % 2, nt * 128 : (nt + 1) * 128],
                            y1[nt][:, jt * 128 : (jt + 1) * 128],
                            ident_bf,
                        )

            # g-passes between f-stages keep the PE busy while LN chains lag
            fstage(0)
            fstage(1)
            filler(8)
            fstage(2)
            gpass([0, 1])
            fstage(3)
            filler(10)
            gpass([2, 3])

            # --- stage G evac: y1T = (Y1Q/Y1S)*y1 in fp8 (or fp16 copy) ---
            with tc.tile_pool(name="y1Tp", bufs=1) as y1Tp:
                y1T = y1Tp.tile([128, KH, S], fp8 if use_fp8 else bf16)
                for jt in range(KH):
                    src = psGt[jt // 2][:, jt % 2, :]
                    if use_fp8:
                        if jt % 2 == 0:
                            nc.scalar.activation(
                                out=y1T[:, jt, :], in_=src, func=AF.Copy,
                                scale=Y1Q / Y1S)
                        else:
                            nc.vector.tensor_scalar_mul(
                                out=y1T[:, jt, :], in0=src,
                                scalar1=Y1Q / Y1S)
                    else:
                        if jt % 2 == 0:
                            nc.scalar.copy(out=y1T[:, jt, :], in_=src)
                        else:
                            nc.vector.tensor_copy(out=y1T[:, jt, :], in_=src)
                es_psG.close()
                es_psF.close()

                # --- stage H: ff = Y1S*relu(y1@W1^T + b1)  (fp16) ---
                es_psHI = ExitStack()
                psH = es_psHI.enter_context(
                    tc.tile_pool(name="psH", bufs=3, space="PSUM"))
                psI = es_psHI.enter_context(
                    tc.tile_pool(name="psI", bufs=2, space="PSUM"))
                # psum scale: fp8: psf = (Y1Q*y1)@(W1SC*W1) ; fp16: Y1S*(y1@W1)
                hsc = Y1S / (Y1Q * W1SC) if use_fp8 else 1.0
                ff1 = []
                for mt in range(KF):
                    psf = psH.tile([128, S], f32, tag="psf", name=f"psH{mt}")
                    if use_fp8:
                        for k2 in range(KH // 2):
                            _mm(nc, psf,
                                w1r[:, 2 * k2 : 2 * k2 + 2,
                                    mt * 128 : (mt + 1) * 128],
                                y1T[:, 2 * k2 : 2 * k2 + 2, :],
                                k2 == 0, k2 == KH // 2 - 1,
                                perf_mode=PM.DoubleRow)
                    else:
                        for kt in range(KH):
                            _mm(nc, psf,
                                w1r[:, kt, mt * 128 : (mt + 1) * 128],
                                y1T[:, kt, :], kt == 0, kt == KH - 1)
                    fft = ffp.tile([128, S], bf16, tag=f"ff{mt}")
                    if apply_b1:
                        nc.scalar.activation(
                            out=fft, in_=psf, func=AF.Relu,
                            bias=b1c_t[:, mt : mt + 1], scale=hsc,
                        )
                    elif mt % 2 == 0:
                        nc.scalar.activation(
                            out=fft, in_=psf, func=AF.Relu, scale=hsc,
                        )
                    else:
                        nc.vector.tensor_scalar(
                            out=fft, in0=psf, scalar1=hsc, scalar2=0.0,
                            op0=OP.mult, op1=OP.max,
                        )
                    ff1.append(fft)

            # --- stage I: psy = ff@W2^T (= Y1S*ff2); LN2(psy + y1) -> out ---
            for nt in range(NT):
                psy = psI.tile([128, H], f32, tag="psy", name=f"psI{nt}")
                for kt in range(KF):
                    lhsT = ff1[kt][:, nt * 128 : (nt + 1) * 128]
                    _mm(nc, psy[:, 0:512], lhsT, w2r[:, kt, 0:512],
                        kt == 0, kt == KF - 1)
                    _mm(nc, psy[:, 512:768], lhsT, w2r[:, kt, 512:768],
                        kt == 0, kt == KF - 1)
                y2 = outp.tile([128, H], bf16, tag="y2")
                with nc.allow_low_precision(reason="pre-LN sum in fp16: 5e-4 rel"):
                    nc.vector.tensor_add(out=y2, in0=psy, in1=y1[nt])
                if apply_b2:
                    nc.vector.tensor_add(out=y2, in0=y2, in1=b2_b)
                mv, rstd = ln_stats(lnp, y2)
                yf = outp.tile([128, H], f32, tag="yf")
                nc.vector.tensor_scalar(out=yf, in0=y2,
                                        scalar1=mv[:, 0:1],
                                        scalar2=rstd[:, 0:1],
                                        op0=OP.subtract, op1=OP.mult)
                if apply_gb:
                    nc.vector.tensor_mul(out=yf, in0=yf, in1=g_b)
                    nc.vector.tensor_add(out=yf, in0=yf, in1=b_b)
                if apply_mask:
                    nc.vector.tensor_scalar_mul(
                        out=yf, in0=yf, scalar1=maskc_t[:, nt : nt + 1]
                    )
                nc.sync.dma_start(
                    out=out_ap[nt * 128 : (nt + 1) * 128, :], in_=yf
                )
            # keep the clock up through the LN2/store tail and epilogue
            filler(40)
            es_psHI.close()
            es_psJ.close()
            es_ctx.close()

    nc.compile()
    return nc


def _sinusoidal_pe():
    pos = np.arange(S, dtype=np.float32)[:, None]
    div = np.exp(
        np.arange(0, H, 2, dtype=np.float32) * (-np.log(10000.0) / H)
    ).astype(np.float32)
    ang = pos * div  # (S, H/2)
    pe = np.stack([np.sin(ang), np.cos(ang)], axis=-1).reshape(S, H)
    return pe.astype(np.float32)


def make_host_data(inputs, use_fp8=True):
    """Host-side constant folding. Returns (shared, per_core, flags)."""
    tok = np.asarray(inputs["token_reps"], dtype=np.float32)
    ids = np.asarray(inputs["span_ids"])
    msk = np.asarray(inputs["span_masks"]).astype(np.float32)
    dq = np.asarray(inputs["dummy_query"], dtype=np.float32)[0, 0]
    ipw = np.asarray(inputs["in_proj_w"], dtype=np.float32)
    ipb = np.asarray(inputs["in_proj_b"], dtype=np.float32)
    out_w = np.asarray(inputs["out_w"], dtype=np.float32)
    out_b = np.asarray(inputs["out_b"], dtype=np.float32)
    lng = np.asarray(inputs["ln_g"], dtype=np.float32)
    lnb = np.asarray(inputs["ln_b"], dtype=np.float32)
    w1 = np.asarray(inputs["ffn_w1"], dtype=np.float32)
    b1 = np.asarray(inputs["ffn_b1"], dtype=np.float32)
    w2 = np.asarray(inputs["ffn_w2"], dtype=np.float32)
    b2 = np.asarray(inputs["ffn_b2"], dtype=np.float32)

    wq, wk, wv = ipw[:H], ipw[H : 2 * H], ipw[2 * H :]
    bq, bk, bv = ipb[:H], ipb[H : 2 * H], ipb[2 * H :]

    q = (dq @ wq.T + bq).astype(np.float32)  # (H,)
    scale = np.float32(1.0 / np.sqrt(HD))
    # Us[:, h] = scale * Wk_h^T q_h  (the constant q.bk_h cancels in softmax)
    Us = np.zeros((H, NH), dtype=np.float32)
    for h in range(NH):
        qh = q[h * HD : (h + 1) * HD]
        Us[:, h] = scale * (wk[h * HD : (h + 1) * HD, :].T @ qh)

    flags = {
        "apply_gb": not (np.all(lng == 1.0) and np.all(lnb == 0.0)),
        "apply_b2": bool(np.any(b2 != 0.0)),
        "apply_b1": bool(np.any(b1 != 0.0)),
        "apply_mask": not np.all(msk == 1.0),
        "use_fp8": use_fp8,
    }

    sel = np.zeros((NH, H), dtype=BF)
    for g in range(NH):
        sel[g, g * HD : (g + 1) * HD] = 1.0

    shared = {
        "wvus": np.ascontiguousarray(
            np.concatenate([wv.T, Us], axis=1).astype(BF)
        ),
        "wot": np.ascontiguousarray(out_w.T.astype(BF)),
        "w2t": np.ascontiguousarray(w2.T.astype(BF)),
        # residual is the RAW dummy query dq, not the projected q
        "addvr": np.ascontiguousarray(
            (out_b + out_w @ bv + dq).astype(BF).reshape(1, H)
        ),
        "sel": sel,
    }
    if use_fp8:
        shared["w1t8"] = np.ascontiguousarray(
            np.clip(w1.T * W1SC, -240, 240).astype(F8)
        )
    else:
        shared["w1t"] = np.ascontiguousarray(w1.T.astype(BF))
    if flags["apply_b1"]:
        # bias folded pre-relu at true scale; evac multiplies psum by hsc
        # first, so bias must be at the Y1S scale
        shared["b1c"] = np.ascontiguousarray(
            (b1 * Y1S).reshape(KF, 128).T, np.float32
        )
    if flags["apply_b2"]:
        shared["b2"] = np.ascontiguousarray(b2 * Y1S, dtype=np.float32)
    if flags["apply_gb"]:
        shared["lng"] = np.ascontiguousarray(lng, dtype=np.float32)
        shared["lnb"] = np.ascontiguousarray(lnb, dtype=np.float32)
        shared["lnb256"] = np.ascontiguousarray(lnb * Y1S, dtype=np.float32)

    pe = _sinusoidal_pe()
    rng = np.arange(S, dtype=np.int64)
    per_core = []
    for b in range(B):
        starts = ids[b, :, 0].astype(np.int64)
        widths = (ids[b, :, 1] - ids[b, :, 0]).astype(np.int64)
        ends = starts + np.where(msk[b] > 0, widths, 0)
        m = ((starts[None, :] <= rng[:, None])
             & (rng[:, None] < ends[None, :])).astype(BF)  # (S, N)
        pc = {
            "xt": np.ascontiguousarray((tok[b] + pe).astype(BF).T),
            "m": np.ascontiguousarray(m),
        }
        if flags["apply_mask"]:
            pc["maskc"] = np.ascontiguousarray(
                msk[b].reshape(NT, 128).T, dtype=np.float32
            )
        per_core.append(pc)
    return shared, per_core, flags


_NC_CACHE = {}


def kernel(**inputs) -> np.ndarray:
    shared, per_core, flags = make_host_data(inputs)
    in_maps = [{**shared, **pc} for pc in per_core]
    key = tuple(sorted(flags.items()))
    if key not in _NC_CACHE:
        _NC_CACHE[key] = build_bass(**flags)
    res = run_bass_kernel_spmd(_NC_CACHE[key], in_maps, core_ids=list(range(B)))
    return np.stack([r["out"] for r in res.results], axis=0)
